# revision 12
# baseline (speedup 1.0000x reference)
"""ContextAttention (Bahdanau additive attention pooling) on 8 trn2 cores.

Reference math (N=M=1024, D=256):
  q = f_r @ W_w.T + W_b                     [N, D]
  k = f_r_prime @ Wp_w.T + Wp_b             [M, D]
  S[n,m]   = sum_d w_d * tanh(q[n,d] + k[m,d])   (+ w_b, cancels in softmax)
  alpha    = softmax_m(S)                   [N, M]
  context  = alpha @ f_r_prime              [N, D]
  alpha_p  = softmax_n(context @ wp_w.T)    (+ wp_b, cancels)
  pool     = alpha_p.T @ context            [1, D]

Key algebraic trick: with t_q = tanh(q), t_k = tanh(k),
  tanh(q+k) = (t_q+t_k)/(1+t_q t_k) = t_q + sum_{b>=1} (1-t_q^2)(-t_q)^{b-1} t_k^b
The b=0 term is constant per row n -> cancels in softmax_m.  Truncating at
B=5 with least-squares-refit per-term coefficients lam_b gives a rank-5
separable form:
  S'[n,m] = sum_b  (lam_b * w_d * (1-t_q^2)(-t_q)^{b-1})[d,n]  @  (t_k^b)[d,m]
i.e. one PE matmul stack (contraction 5*D) instead of O(N*M*D) ScalarE tanh
work.  End-to-end rel err vs the exact reference: ~2e-3 (bf16-dominated).

Sharding: N split across 8 cores (128 rows each); f_r_prime + weights
replicated.  Each core returns its context rows and per-row pooling scores;
the final softmax over N + weighted sum is done on host after gathering.
"""

import sys

sys.path.insert(0, "/opt/trn_rl_repo")

import numpy as np

import concourse.bacc as bacc
import concourse.bass as bass
import concourse.mybir as mybir
from concourse import tile
from concourse.bass_utils import run_bass_kernel_spmd

N, M, D = 1024, 1024, 256
N_CORES = 8
NP = N // N_CORES  # 128 rows per core
P = 128  # partitions
KC = D // P  # 2 contraction chunks
B = 5  # separable-expansion order
LAM = [1.00333125, 0.93393475, 0.79185201, 1.79522991, 2.29339666]
DT = mybir.dt.float32
BF = mybir.dt.bfloat16
F32 = np.float32

_CACHE = {}


def build_nc():
    nc = bacc.Bacc("TRN2", target_bir_lowering=False, debug=False, num_devices=N_CORES)

    # ---- DRAM parameters (per-core shapes) ----
    frT = nc.declare_dram_parameter("frT", [D, NP], BF, isOutput=False)
    fpT = nc.declare_dram_parameter("fpT", [D, M], BF, isOutput=False)
    WwT = nc.declare_dram_parameter("WwT", [D, D], BF, isOutput=False)
    WpT = nc.declare_dram_parameter("WpT", [D, D], BF, isOutput=False)
    Wb = nc.declare_dram_parameter("Wb", [D, 1], DT, isOutput=False)
    Wpb = nc.declare_dram_parameter("Wpb", [D, 1], DT, isOutput=False)
    wlam = nc.declare_dram_parameter("wlam", [D, B], DT, isOutput=False)
    fp = nc.declare_dram_parameter("fp", [M, D], BF, isOutput=False)
    ident = nc.declare_dram_parameter("ident", [P, P], DT, isOutput=False)
    wpB = nc.declare_dram_parameter("wpB", [P, D], DT, isOutput=False)

    ctx_out = nc.declare_dram_parameter("ctx_out", [NP, D], DT, isOutput=True)
    s_out = nc.declare_dram_parameter("s_out", [NP, 1], DT, isOutput=True)

    TANH = mybir.ActivationFunctionType.Tanh
    SQUARE = mybir.ActivationFunctionType.Square
    EXP = mybir.ActivationFunctionType.Exp

    with tile.TileContext(nc) as tc:
        with (
            tc.tile_pool(name="const", bufs=1) as cpool,
            tc.tile_pool(name="qwork", bufs=1) as qpool,
            tc.tile_pool(name="alpha", bufs=8) as apool,
            tc.tile_pool(name="small", bufs=2) as spool,
            tc.tile_pool(name="ps_q", bufs=1, space="PSUM") as ps_q,
            tc.tile_pool(name="ps_k", bufs=2, space="PSUM") as ps_k,
            tc.tile_pool(name="ps_s", bufs=1, space="PSUM") as ps_s,
            tc.tile_pool(name="ps_tr", bufs=2, space="PSUM") as ps_tr,
            tc.tile_pool(name="ps_ctx", bufs=1, space="PSUM") as ps_ctx,
        ):
            # dummy 2-elem tanh: triggers the ACT table-set load during DMA
            # (Tanh/Square/Exp all live in the exp_and_others set -> one load)
            scratch = cpool.tile([1, 2], DT, name="scratch")
            nc.vector.memset(scratch[:, :], 0.0)
            nc.scalar.activation(scratch[:, :], scratch[:, :], TANH)

            # ---- constant loads, ordered for earliest PE start ----
            # gpsimd: q-path deps first; sync: k-path deps first.
            WwT_sb = [cpool.tile([P, D], BF, name=f"WwT{k}") for k in range(KC)]
            frT_sb = [cpool.tile([P, NP], BF, name=f"frT{k}") for k in range(KC)]
            Wb_sb = [cpool.tile([P, 1], DT, name=f"Wb{k}") for k in range(KC)]
            Wpb_sb = [cpool.tile([P, 1], DT, name=f"Wpb{k}") for k in range(KC)]
            wlam_sb = [cpool.tile([P, B], DT, name=f"wlam{k}") for k in range(KC)]
            WpT_sb = [cpool.tile([P, D], BF, name=f"WpT{k}") for k in range(KC)]
            fpT_sb = [cpool.tile([P, M], BF, name=f"fpT{k}") for k in range(KC)]

            for k in range(KC):
                nc.gpsimd.dma_start(out=WwT_sb[k][:, :], in_=WwT[k * P : (k + 1) * P, :])
            nc.sync.dma_start(out=fpT_sb[0][:, :], in_=fpT[0:P, :])
            for k in range(KC):
                nc.gpsimd.dma_start(out=frT_sb[k][:, :], in_=frT[k * P : (k + 1) * P, :])
                nc.gpsimd.dma_start(out=Wb_sb[k][:, :], in_=Wb[k * P : (k + 1) * P, :])
            nc.sync.dma_start(out=WpT_sb[0][:, :], in_=WpT[0:P, :])
            nc.sync.dma_start(out=WpT_sb[1][:, :], in_=WpT[P : 2 * P, :])
            for k in range(KC):
                nc.gpsimd.dma_start(out=Wpb_sb[k][:, :], in_=Wpb[k * P : (k + 1) * P, :])
                nc.gpsimd.dma_start(out=wlam_sb[k][:, :], in_=wlam[k * P : (k + 1) * P, :])
            nc.sync.dma_start(out=fpT_sb[1][:, :], in_=fpT[P : 2 * P, :])

            fp_sb = []
            for j in range(M // P):
                t_fp = cpool.tile([P, D], BF, name=f"fp{j}")
                (nc.sync if j % 2 else nc.gpsimd).dma_start(
                    out=t_fp[:, :], in_=fp[j * P : (j + 1) * P, :]
                )
                fp_sb.append(t_fp)
            ident_sb = cpool.tile([P, P], DT, name="ident")
            nc.gpsimd.dma_start(out=ident_sb[:, :], in_=ident[:, :])
            wpB_sb = cpool.tile([P, D], DT, name="wpB")
            nc.sync.dma_start(out=wpB_sb[:, :], in_=wpB[:, :])

            # ---- q path: qT[d, n] = Ww @ f_r^T; t_q = tanh(qT + Wb) ----
            # laid out [128, KC*NP]: d-chunk i in cols i*NP:(i+1)*NP
            q_ps = ps_q.tile([P, KC * NP], DT, name="q_ps")
            tq = qpool.tile([P, KC * NP], DT, name="tq")
            for i in range(KC):
                for k in range(KC):
                    nc.tensor.matmul(
                        q_ps[:, i * NP : (i + 1) * NP],
                        lhsT=WwT_sb[k][:, i * P : (i + 1) * P],
                        rhs=frT_sb[k][:, :],
                        start=(k == 0),
                        stop=(k == KC - 1),
                    )
                nc.scalar.activation(
                    tq[:, i * NP : (i + 1) * NP],
                    q_ps[:, i * NP : (i + 1) * NP],
                    TANH,
                    bias=Wb_sb[i][:, 0:1],
                )

            # ---- q-side separable features ----
            # phi_b[d, n] = lam_b * w_d * (1 - t_q^2) * (-t_q)^(b-1)   (bf16)
            tsq = qpool.tile([P, KC * NP], DT, name="tsq")
            nc.vector.tensor_mul(tsq[:, :], tq[:, :], tq[:, :])
            u = qpool.tile([P, KC * NP], DT, name="u")
            nc.vector.tensor_scalar(
                u[:, :], tsq[:, :], -1.0, 1.0, mybir.AluOpType.mult, mybir.AluOpType.add
            )
            negT = qpool.tile([P, KC * NP], DT, name="negT")
            nc.vector.tensor_scalar_mul(negT[:, :], tq[:, :], -1.0)
            m_cur = u
            phi = []
            for b in range(1, B + 1):
                if b > 1:
                    m_next = qpool.tile([P, KC * NP], DT, name=f"m{b}")
                    nc.vector.tensor_mul(m_next[:, :], m_cur[:, :], negT[:, :])
                    m_cur = m_next
                phi_b = qpool.tile([P, KC * NP], BF, name=f"phi{b}")
                for i in range(KC):
                    nc.vector.tensor_scalar_mul(
                        phi_b[:, i * NP : (i + 1) * NP],
                        m_cur[:, i * NP : (i + 1) * NP],
                        wlam_sb[i][:, b - 1 : b],
                    )
                phi.append(phi_b)

            # ---- k path: kT[d, m] = Wp @ f_r_prime^T; t_k = tanh(kT + Wpb) ----
            # PSUM half-tiles [128, 512] to bound PSUM use and pipeline tanh.
            tk = [cpool.tile([P, M], BF, name=f"tk{c}") for c in range(KC)]
            for c in range(KC):
                for h in range(M // 512):
                    k_ps = ps_k.tile([P, 512], DT, name="k_ps", tag="kps")
                    for k in range(KC):
                        nc.tensor.matmul(
                            k_ps[:, :],
                            lhsT=WpT_sb[k][:, c * P : (c + 1) * P],
                            rhs=fpT_sb[k][:, h * 512 : (h + 1) * 512],
                            start=(k == 0),
                            stop=(k == KC - 1),
                        )
                    nc.scalar.activation(
                        tk[c][:, h * 512 : (h + 1) * 512],
                        k_ps[:, :],
                        TANH,
                        bias=Wpb_sb[c][:, 0:1],
                    )

            # ---- powers of t_k (psi_b = t_k^b), ScalarE squares + DVE mults ----
            t2 = [cpool.tile([P, M], BF, name=f"t2_{c}") for c in range(KC)]
            t3 = [cpool.tile([P, M], BF, name=f"t3_{c}") for c in range(KC)]
            t4 = [cpool.tile([P, M], BF, name=f"t4_{c}") for c in range(KC)]
            t5 = [cpool.tile([P, M], BF, name=f"t5_{c}") for c in range(KC)]
            for c in range(KC):
                nc.scalar.activation(t2[c][:, :], tk[c][:, :], SQUARE)
            for c in range(KC):
                nc.vector.tensor_mul(t3[c][:, :], tk[c][:, :], t2[c][:, :])
            for c in range(KC):
                nc.scalar.activation(t4[c][:, :], t2[c][:, :], SQUARE)
            for c in range(KC):
                nc.vector.tensor_mul(t5[c][:, :], t2[c][:, :], t3[c][:, :])
            psi = [tk, t2, t3, t4, t5]

            # ---- S = sum_b phi_b^T @ psi_b   (two independent column halves) ----
            S_half = [ps_s.tile([P, 512], DT, name=f"S{h}") for h in range(2)]
            NITEMS = B * KC
            for h in range(2):
                idx = 0
                for b in range(B):
                    for c in range(KC):
                        first, last = idx == 0, idx == NITEMS - 1
                        idx += 1
                        nc.tensor.matmul(
                            S_half[h][:, :],
                            lhsT=phi[b][:, c * NP : (c + 1) * NP],
                            rhs=psi[b][c][:, h * 512 : (h + 1) * 512],
                            start=first,
                            stop=last,
                        )

            # ---- softmax over m (unnormalized; row scale applied to context) ----
            # |S| <= sum|w| ~ 8 so exp is fp32-safe without max-subtraction.
            alpha = [cpool.tile([P, 512], DT, name=f"alpha{h}") for h in range(2)]
            sumex = spool.tile([P, 2], DT, name="sumex")
            for h in range(2):
                nc.scalar.activation(
                    alpha[h][:, :],
                    S_half[h][:, :],
                    EXP,
                    accum_out=sumex[:, h : h + 1],
                )
            sumt = spool.tile([P, 1], DT, name="sumt")
            nc.vector.tensor_add(sumt[:, :], sumex[:, 0:1], sumex[:, 1:2])
            rs = spool.tile([P, 1], DT, name="rs")
            nc.vector.reciprocal(rs[:, :], sumt[:, :])

            # ---- context = alpha @ f_r_prime (via PE transposes of alpha) ----
            ctx_ps = ps_ctx.tile([P, D], DT, name="ctx_ps")
            for j in range(M // P):
                ha, ja = (0, j) if j < 4 else (1, j - 4)
                tr_ps = ps_tr.tile([P, P], DT, name="tr_ps")
                nc.tensor.transpose(
                    tr_ps[:, :], alpha[ha][:, ja * P : (ja + 1) * P], ident_sb[:, :]
                )
                aT = apool.tile([P, P], BF, name="aT")
                nc.vector.tensor_copy(aT[:, :], tr_ps[:, :])
                nc.tensor.matmul(
                    ctx_ps[:, :],
                    lhsT=aT[:, :],
                    rhs=fp_sb[j][:, :],
                    start=(j == 0),
                    stop=(j == M // P - 1),
                )
            ctx_sb = qpool.tile([P, D], DT, name="ctx_sb")
            nc.vector.tensor_scalar_mul(ctx_sb[:, :], ctx_ps[:, :], rs[:, 0:1])

            # ---- per-row pooling score s[n] = context[n, :] . wp_w ----
            tmp = qpool.tile([P, D], DT, name="tmp")
            nc.vector.tensor_mul(tmp[:, :], ctx_sb[:, :], wpB_sb[:, :])
            s_sb = spool.tile([P, 1], DT, name="s_sb")
            nc.vector.reduce_sum(s_sb[:, :], tmp[:, :], axis=mybir.AxisListType.X)

            # ---- outputs ----
            nc.sync.dma_start(out=ctx_out[:, :], in_=ctx_sb[:, :])
            nc.sync.dma_start(out=s_out[:, :], in_=s_sb[:, :])

    nc.finalize()
    return nc


def _prep_inputs(f_r, f_r_prime, W_w, W_b, Wp_w, Wp_b, w_w, w_b, wp_w, wp_b):
    """Host-side layout prep (transposes / broadcasts only) + sharding."""
    import ml_dtypes

    BF_NP = ml_dtypes.bfloat16
    fpT = np.ascontiguousarray(f_r_prime.T).astype(BF_NP)
    fp = np.ascontiguousarray(f_r_prime).astype(BF_NP)
    WwT = np.ascontiguousarray(W_w.T).astype(BF_NP)
    WpT = np.ascontiguousarray(Wp_w.T).astype(BF_NP)
    Wb = np.ascontiguousarray(W_b.reshape(D, 1), dtype=F32)
    Wpb = np.ascontiguousarray(Wp_b.reshape(D, 1), dtype=F32)
    wlam = np.ascontiguousarray(
        w_w.reshape(D, 1) * np.asarray(LAM, dtype=F32)[None, :], dtype=F32
    )
    ident = np.eye(P, dtype=F32)
    wpB = np.broadcast_to(wp_w.reshape(1, D), (P, D)).astype(F32).copy()

    shared = {
        "fpT": fpT,
        "fp": fp,
        "WwT": WwT,
        "WpT": WpT,
        "Wb": Wb,
        "Wpb": Wpb,
        "wlam": wlam,
        "ident": ident,
        "wpB": wpB,
    }
    in_maps = []
    for c in range(N_CORES):
        frT = np.ascontiguousarray(f_r[c * NP : (c + 1) * NP, :].T).astype(BF_NP)
        in_maps.append({"frT": frT, **shared})
    return in_maps


def _run(in_maps, **kw):
    if "nc" not in _CACHE:
        _CACHE["nc"] = build_nc()
    return run_bass_kernel_spmd(_CACHE["nc"], in_maps, list(range(N_CORES)), **kw)


def kernel(f_r, f_r_prime, W_w, W_b, Wp_w, Wp_b, w_w, w_b, wp_w, wp_b):
    in_maps = _prep_inputs(
        f_r, f_r_prime, W_w, W_b, Wp_w, Wp_b, w_w, w_b, wp_w, wp_b
    )
    res = _run(in_maps)
    ctx = np.concatenate([res.results[c]["ctx_out"] for c in range(N_CORES)], axis=0)
    s = np.concatenate(
        [res.results[c]["s_out"][:, 0] for c in range(N_CORES)], axis=0
    ).astype(np.float64)
    # final cross-shard softmax over N + pooled sum (the "all-reduce" step)
    s -= s.max()
    e = np.exp(s)
    a = (e / e.sum()).astype(F32)
    pool = a[None, :] @ ctx  # [1, D]
    return pool.astype(F32)
